# revision 1
# baseline (speedup 1.0000x reference)
"""Deformable Conv v2 (DCNv2) Trainium2 Bass kernel.

Problem: x[4,256,64,64], w_off[27,256,3,3], b_off[27], w_conv[256,256,3,3]
  -> out[4,256,64,64]  (offset conv + bilinear sampling + 9-point GEMM)

Sharding: 8 cores = 4 batches x 2 H-halves. Each core computes out for its
(batch, 32-row half): 2048 output pixels.

Per-core pipeline (single SPMD program):
  1. offset conv as 9 shifted-tap GEMMs on a C-major padded bf16 slice
  2. PE-transpose om to pixel-major, compute bilinear coefs (fp32) + indices
  3. dma_gather of overlapping [2C] rows (x-corner pairs) from the padded
     channels-last bf16 image in DRAM
  4. per (k, pixel-tile): 4 tensor-scalar corner mults (bf16) + paired add;
     y-corner sum folded into PE transpose with PSUM accumulation
  5. main GEMM: out[o,p] += W_k[c,o].T @ val_k[c,p], 18 K-tiles, bf16
"""

import numpy as np
import ml_dtypes

import concourse.bacc as bacc
import concourse.bass as bass
import concourse.mybir as mybir
import concourse.tile as tile
from concourse.bass_utils import run_bass_kernel_spmd

F32 = mybir.dt.float32
BF16 = mybir.dt.bfloat16
I16 = mybir.dt.int16
OP = mybir.AluOpType
AF = mybir.ActivationFunctionType

B, C, H, W, O, K = 4, 256, 64, 64, 256, 9
PADR = 2                      # zero-pad ring width
Hp, Wp = H + 2 * PADR, W + 2 * PADR            # 68, 68
NPIX = 2048                   # output pixels per core (32 rows x 64)
NT = NPIX // 128              # 16 pixel tiles
OMW = 34 * Wp                 # om computed on full 68-wide rows: 2312? (see below)
N_CORES = 8

Bb = ml_dtypes.bfloat16

# om is computed for 32 output rows on full 68-wide (incl pad) columns
OMCOLS = 32 * Wp              # 2176
XCM_COLS = 36 * Wp            # 2448 (om conv input slice: rows h0..h0+35)
OM_BLOCKS = [(0, 512), (512, 512), (1024, 512), (1536, 512), (2048, 128)]


STAGE = 5


def build_program():
    nc = bacc.Bacc("TRN2", target_bir_lowering=False, debug=False,
                   num_devices=N_CORES)
    xcl_d = nc.dram_tensor("xcl", [Hp * Wp * C], BF16, kind="ExternalInput")
    xcm_d = nc.dram_tensor("xcm", [2, 128, XCM_COLS], BF16, kind="ExternalInput")
    woff_d = nc.dram_tensor("woff", [2, 128, 9 * 27], BF16, kind="ExternalInput")
    wcv_d = nc.dram_tensor("wcv", [2, 128, 2 * 9 * 128], BF16, kind="ExternalInput")
    gyk_d = nc.dram_tensor("gyk", [128, 144], F32, kind="ExternalInput")
    gxk_d = nc.dram_tensor("gxk", [128, 144], F32, kind="ExternalInput")
    boff_d = nc.dram_tensor("boff", [27, 1], F32, kind="ExternalInput")
    identb_d = nc.dram_tensor("identb", [128, 128], BF16, kind="ExternalInput")
    identf_d = nc.dram_tensor("identf", [32, 32], F32, kind="ExternalInput")
    y_d = nc.dram_tensor("y", [2, 128, NPIX], F32, kind="ExternalOutput")

    scratch = nc.dram_tensor("idx_scratch", [18 * NPIX], I16, kind="Internal")

    with tile.TileContext(nc) as tc:
        _emit(nc, tc, xcl_d, xcm_d, woff_d, wcv_d, gyk_d, gxk_d, boff_d,
              identb_d, identf_d, y_d, scratch)
    nc.compile()
    return nc


def _emit(nc, tc, xcl_d, xcm_d, woff_d, wcv_d, gyk_d, gxk_d, boff_d,
          identb_d, identf_d, y_d, scratch):
    with tc.tile_pool(name="const", bufs=1) as cpool:
        _emit_body(nc, tc, cpool, xcl_d, xcm_d, woff_d, wcv_d, gyk_d, gxk_d,
                   boff_d, identb_d, identf_d, y_d, scratch)


def _finish_stub(nc, tc, pool, y_d):
    z = pool.tile([128, NPIX], F32, tag="zstub", name="zstub")
    nc.vector.memset(z[:, :], 0.0)
    nc.sync.dma_start(y_d.ap()[0], z[:, :])
    nc.sync.dma_start(y_d.ap()[1], z[:, :])


def _emit_body(nc, tc, cpool, xcl_d, xcm_d, woff_d, wcv_d, gyk_d, gxk_d,
               boff_d, identb_d, identf_d, y_d, scratch):
    # --- persistent constants ---
    wcv = [cpool.tile([128, 2 * 9 * 128], BF16, tag=f"wcv{ct}", name=f"wcv{ct}") for ct in range(2)]
    for ct in range(2):
        nc.sync.dma_start(wcv[ct][:, :], wcv_d.ap()[ct])
    woff = [cpool.tile([128, 9 * 27], BF16, tag=f"woff{ct}", name=f"woff{ct}") for ct in range(2)]
    for ct in range(2):
        nc.sync.dma_start(woff[ct][:, :], woff_d.ap()[ct])
    gyk = cpool.tile([128, 144], F32, tag="gyk", name="gyk")
    nc.sync.dma_start(gyk[:, :], gyk_d.ap()[:, :])
    gxk = cpool.tile([128, 144], F32, tag="gxk", name="gxk")
    nc.sync.dma_start(gxk[:, :], gxk_d.ap()[:, :])
    boff = cpool.tile([27, 1], F32, tag="boff", name="boff")
    nc.sync.dma_start(boff[:, :], boff_d.ap()[:, :])
    identb = cpool.tile([128, 128], BF16, tag="identb", name="identb")
    nc.sync.dma_start(identb[:, :], identb_d.ap()[:, :])
    identf = cpool.tile([32, 32], F32, tag="identf", name="identf")
    nc.sync.dma_start(identf[:, :], identf_d.ap()[:, :])

    # persistent: corner coefs, wrapped indices
    c00 = cpool.tile([128, 144], F32, tag="c00", name="c00")
    c01 = cpool.tile([128, 144], F32, tag="c01", name="c01")
    c10 = cpool.tile([128, 144], F32, tag="c10", name="c10")
    c11 = cpool.tile([128, 144], F32, tag="c11", name="c11")
    iw = cpool.tile([128, 18 * 128], I16, tag="iw", name="iw")

    # ---------------- Phase A: offset conv + coefs (scoped pools) ---------
    with tc.tile_pool(name="early", bufs=1) as epool, \
         tc.tile_pool(name="om_ps", bufs=2, space="PSUM") as om_ps, \
         tc.tile_pool(name="omp_ps", bufs=1, space="PSUM") as omp_ps:
        xcm = [epool.tile([128, XCM_COLS], BF16, tag=f"xcm{ct}", name=f"xcm{ct}") for ct in range(2)]
        for ct in range(2):
            nc.sync.dma_start(xcm[ct][:, :], xcm_d.ap()[ct])

        if STAGE < 1:
            _finish_stub(nc, tc, cpool, y_d)
            return
        om_s = epool.tile([27, OMCOLS], F32, tag="om_s", name="om_s")
        for nboff, nbsz in OM_BLOCKS:
            omp = om_ps.tile([27, 512], F32, tag="omps", name="omps")
            first = True
            for tap in range(9):
                ky, kx = tap // 3, tap % 3
                toff = (ky + 1) * Wp + kx - 1
                for ct in range(2):
                    nc.tensor.matmul(
                        omp[:, 0:nbsz],
                        woff[ct][:, tap * 27:(tap + 1) * 27],
                        xcm[ct][:, toff + nboff: toff + nboff + nbsz],
                        start=first, stop=(tap == 8 and ct == 1),
                    )
                    first = False
            nc.scalar.activation(om_s[:, nboff:nboff + nbsz], omp[:, 0:nbsz],
                                 AF.Identity, bias=boff[:, 0:1])

        if STAGE < 2:
            _finish_stub(nc, tc, cpool, y_d)
            return
        # om -> pixel-major via PE transpose (compact valid pixels first:
        # matmul operands must have a single free dim)
        om_v = epool.tile([27, NPIX], F32, tag="om_v", name="om_v")
        om_h = om_s[:, :]
        nc.vector.tensor_copy(
            om_v[:, :],
            bass.AP(om_h.tensor, om_h.offset + 2,
                    [list(om_h.ap[0]), [Wp, 32], [1, 64]]))
        omp_pm = omp_ps.tile([128, NT * 27], F32, tag="omppm", name="omppm")
        for t in range(NT):
            nc.tensor.matmul(omp_pm[:, 27 * t:27 * (t + 1)],
                             om_v[:, 128 * t:128 * (t + 1)],
                             identf[0:27, 0:27], is_transpose=True,
                             start=True, stop=True)
        omp_s = epool.tile([128, NT * 27], F32, tag="omp_s", name="omp_s")
        nc.scalar.copy(omp_s[:, :], omp_pm[:, :])

        # --- coef pipeline (pixel-major [128, 16, 9] strided views) ---
        base = omp_s[:, :]
        p0 = list(base.ap[0])

        def omview(ch_off, ch_step):
            return bass.AP(base.tensor, base.offset + ch_off,
                           [p0, [27, NT], [ch_step, 9]])

        def wtile(tag):
            return epool.tile([128, 144], F32, tag=tag, name=tag)

        py = wtile("py")
        nc.vector.tensor_tensor(py[:, :], omview(0, 2), gyk[:, :], OP.add)
        px = wtile("px")
        nc.vector.tensor_tensor(px[:, :], omview(1, 2), gxk[:, :], OP.add)

        # floor via +16-bias cast roundtrip (correct for trunc OR round-to-
        # nearest casts; bias keeps the operand positive, clamp absorbs it).
        I32 = mybir.dt.int32
        BIAS = 16.0

        def floor_frac(p, pre):
            pt = wtile(pre + "t")
            nc.vector.tensor_scalar(pt[:, :], p[:, :], BIAS, None, OP.add)
            pi = epool.tile([128, 144], I32, tag=pre + "i", name=pre + "i")
            nc.vector.tensor_copy(pi[:, :], pt[:, :])
            pf = wtile(pre + "f")
            nc.vector.tensor_copy(pf[:, :], pi[:, :])
            gg = wtile(pre + "g")
            nc.vector.tensor_tensor(gg[:, :], pf[:, :], pt[:, :], OP.is_gt)
            fb = wtile(pre + "fb")   # floor(p)+BIAS
            nc.vector.tensor_tensor(fb[:, :], pf[:, :], gg[:, :], OP.subtract)
            fr = wtile(pre + "fr")   # frac(p)
            nc.vector.tensor_tensor(fr[:, :], pt[:, :], fb[:, :], OP.subtract)
            return fb, fr

        y0b, wy = floor_frac(py, "y")
        x0b, wx = floor_frac(px, "x")
        # clamp (still biased by +16): [-2, H] -> [14, H+16]
        nc.vector.tensor_scalar(y0b[:, :], y0b[:, :], 14.0, float(H) + BIAS,
                                OP.max, OP.min)
        nc.vector.tensor_scalar(x0b[:, :], x0b[:, :], 14.0, float(W) + BIAS,
                                OP.max, OP.min)
        # idx = 68*(y0+2) + x0+2 = 68*y0b + x0b - 966
        idxf = epool.tile([128, 2, 144], F32, tag="idxf", name="idxf")
        nc.vector.tensor_scalar(idxf[:, 0, :], y0b[:, :], float(Wp), -966.0,
                                OP.mult, OP.add)
        nc.vector.tensor_tensor(idxf[:, 0, :], idxf[:, 0, :], x0b[:, :], OP.add)
        nc.vector.tensor_scalar(idxf[:, 1, :], idxf[:, 0, :], float(Wp), None,
                                OP.add)
        idx16 = epool.tile([128, 2, 144], I16, tag="idx16", name="idx16")
        nc.vector.tensor_copy(idx16[:, :, :], idxf[:, :, :])

        msk = wtile("msk")
        nc.scalar.activation(msk[:, :], omview(18, 1), AF.Sigmoid)
        b1 = wtile("b1")
        nc.vector.tensor_tensor(b1[:, :], wy[:, :], msk[:, :], OP.mult)
        b0 = wtile("b0")
        nc.vector.tensor_tensor(b0[:, :], msk[:, :], b1[:, :], OP.subtract)
        nc.vector.tensor_tensor(c01[:, :], b0[:, :], wx[:, :], OP.mult)
        nc.vector.tensor_tensor(c00[:, :], b0[:, :], c01[:, :], OP.subtract)
        nc.vector.tensor_tensor(c11[:, :], b1[:, :], wx[:, :], OP.mult)
        nc.vector.tensor_tensor(c10[:, :], b1[:, :], c11[:, :], OP.subtract)

        # --- index wrap to [16, n/16] layout via DRAM bounce ---
        # ks = y*9+k slice index (18 slices: y0 rows then y1 rows)
        idx16b = epool.tile([128, 288], I16, tag="idx16b", name="idx16b")
        for y in range(2):
            nc.vector.tensor_copy(
                idx16b[:, 144 * y:144 * (y + 1)].rearrange(
                    "p (k t) -> p k t", t=NT),
                idx16[:, y, :].rearrange("p (t k) -> p k t", k=9))
        nc.sync.dma_start(
            scratch.ap().rearrange("(ks t r) -> r ks t", r=128, t=NT),
            idx16b[:, :].rearrange("p (ks t) -> p ks t", t=NT))
        # indices must be replicated into each 16-partition block (one per
        # GPSIMD Q7 core)
        for rep in range(8):
            nc.sync.dma_start(
                iw[16 * rep:16 * rep + 16, :].rearrange("p (k j) -> p k j", k=18),
                scratch.ap().rearrange("(k j q) -> q k j", q=16, j=128))

    if STAGE < 3:
        _finish_stub(nc, tc, cpool, y_d)
        return
    # ---------------- Phase B: gather / apply / transpose / GEMM ----------
    xcl_h = xcl_d  # flat [Hp*Wp*C]
    win0 = bass.AP(xcl_h, 0, [[C, Hp * Wp - 1], [1, 2 * C]])

    with tc.tile_pool(name="val", bufs=9) as vpool, \
         tc.tile_pool(name="g", bufs=2) as gpool, \
         tc.tile_pool(name="ab", bufs=4) as apool, \
         tc.tile_pool(name="outs", bufs=2) as opool, \
         tc.tile_pool(name="gemm_ps", bufs=4, space="PSUM") as gemm_ps, \
         tc.tile_pool(name="tp_ps", bufs=4, space="PSUM") as tp_ps:

        vals = []
        gps0 = [gemm_ps.tile([128, 512], F32, tag="gps", name="gps") for _ in range(4)]

        def emit_gemm_k(gps, k, ot):
            for ct in range(2):
                for nb in range(4):
                    nc.tensor.matmul(
                        gps[nb][:, :],
                        wcv[ct][:, (ot * 9 + k) * 128:(ot * 9 + k + 1) * 128],
                        vals[k][:, ct, nb * 512:(nb + 1) * 512],
                        start=(k == 0 and ct == 0),
                        stop=(k == 8 and ct == 1),
                    )

        for k in range(9):
            g0 = gpool.tile([128, NT, 2 * C], BF16, tag="g0", name="g0")
            g1 = gpool.tile([128, NT, 2 * C], BF16, tag="g1", name="g1")
            ik0 = gpool.tile([128, 128], I16, tag="ik0", name="ik0")
            nc.vector.tensor_copy(
                ik0[:, :], iw[:, 128 * k:128 * (k + 1)])
            ik1 = gpool.tile([128, 128], I16, tag="ik1", name="ik1")
            nc.vector.tensor_copy(
                ik1[:, :], iw[:, 128 * (9 + k):128 * (10 + k)])
            for q in range(4):   # 512-idx calls: idx positions [512q, 512q+512)
                # positions i at iw[i%16, i//16]: block q -> cols [32q, 32q+32)
                nc.gpsimd.dma_gather(
                    out_ap=g0[:, 4 * q:4 * q + 4, :], in_ap=win0,
                    idxs_ap=ik0[:, 32 * q:32 * q + 32],
                    num_idxs=512, num_idxs_reg=512,
                    elem_size=2 * C, elem_step=C)
                nc.gpsimd.dma_gather(
                    out_ap=g1[:, 4 * q:4 * q + 4, :], in_ap=win0,
                    idxs_ap=ik1[:, 32 * q:32 * q + 32],
                    num_idxs=512, num_idxs_reg=512,
                    elem_size=2 * C, elem_step=C)

            if STAGE < 4:
                continue
            val = vpool.tile([128, 2, NPIX], BF16, tag="val", name="val")
            vals.append(val)
            for half in range(4):      # 4 pixel-quads of 4 tiles each
                tp = [tp_ps.tile([128, 512], BF16, tag="tp", name="tp") for _ in range(2)]
                for t in range(4 * half, 4 * half + 4):
                    col = t * 9 + k
                    mb = apool.tile([128, 4, C], BF16, tag="mb", name="mb")
                    nc.vector.tensor_scalar(mb[:, 0, :], g0[:, t, 0:C],
                                            c00[:, col:col + 1], None, OP.mult)
                    nc.scalar.activation(mb[:, 2, :], g0[:, t, C:2 * C],
                                         AF.Copy, scale=c01[:, col:col + 1])
                    nc.vector.tensor_scalar(mb[:, 1, :], g1[:, t, 0:C],
                                            c10[:, col:col + 1], None, OP.mult)
                    nc.vector.tensor_scalar(mb[:, 3, :], g1[:, t, C:2 * C],
                                            c11[:, col:col + 1], None, OP.mult)
                    ab = apool.tile([128, 2, C], BF16, tag="ab", name="ab")
                    nc.vector.tensor_tensor(ab[:, :, :], mb[:, 0:2, :],
                                            mb[:, 2:4, :], OP.add)
                    vt = apool.tile([128, C], BF16, tag="vt", name="vt")
                    nc.vector.tensor_tensor(vt[:, :], ab[:, 0, :],
                                            ab[:, 1, :], OP.add)
                    # PE transpose pixel-major val tile -> C-major (PSUM)
                    sl = slice((t % 4) * 128, (t % 4) * 128 + 128)
                    for ch in range(2):
                        nc.tensor.matmul(tp[ch][:, sl],
                                         vt[:, ch * 128:(ch + 1) * 128],
                                         identb[:, :], is_transpose=True,
                                         start=True, stop=True)
                for ch in range(2):
                    nc.scalar.copy(val[:, ch, half * 512:(half + 1) * 512],
                                   tp[ch][:, :])
            if STAGE >= 5 and k >= 1:
                emit_gemm_k(gps0, k - 1, ot=0)

        if STAGE < 5:
            _finish_stub(nc, tc, cpool, y_d)
            return
        emit_gemm_k(gps0, 8, ot=0)
        out0 = opool.tile([128, NPIX], F32, tag="outs", name="outs")
        for nb in range(4):
            nc.scalar.copy(out0[:, nb * 512:(nb + 1) * 512], gps0[nb][:, :])
        nc.sync.dma_start(y_d.ap()[0], out0[:, :])

        gps1 = [gemm_ps.tile([128, 512], F32, tag="gps", name="gps") for _ in range(4)]
        for k in range(9):
            emit_gemm_k(gps1, k, ot=1)
        out1 = opool.tile([128, NPIX], F32, tag="outs", name="outs")
        for nb in range(4):
            nc.scalar.copy(out1[:, nb * 512:(nb + 1) * 512], gps1[nb][:, :])
        nc.sync.dma_start(y_d.ap()[1], out1[:, :])


# ---------------------------------------------------------------------------
# Host side
# ---------------------------------------------------------------------------

def make_core_inputs(x, w_off, b_off, w_conv, core):
    b, s = core // 2, core % 2
    h0 = 32 * s
    xp = np.zeros((C, Hp, Wp), np.float32)
    xp[:, PADR:PADR + H, PADR:PADR + W] = x[b]
    xcl = np.ascontiguousarray(xp.transpose(1, 2, 0)).reshape(-1).astype(BbArr)
    xcm = np.ascontiguousarray(
        xp.reshape(C, Hp * Wp)[:, h0 * Wp: h0 * Wp + XCM_COLS]
    ).astype(BbArr).reshape(2, 128, XCM_COLS)

    # w_off [27, C, 3, 3] -> [2, 128, 9*27]: [ct, c, tap*27+oc]
    wof = w_off.transpose(1, 2, 3, 0).reshape(2, 128, 9, 27).reshape(2, 128, 243)
    wof = np.ascontiguousarray(wof).astype(BbArr)
    # w_conv [O, C, 3, 3] -> [2(ct), 128(c), (ot*9+k)*128+o]
    wc = w_conv.reshape(2, 128, C, 9).transpose(2, 0, 3, 1)   # [c, ot, k, o128]
    wc = np.ascontiguousarray(wc.reshape(2, 128, 2 * 9 * 128)).astype(BbArr)

    r = np.arange(128)[:, None, None]
    t = np.arange(NT)[None, :, None]
    kk = np.arange(9)[None, None, :]
    gyk = (h0 + 2 * t + r // 64 - 1 + kk // 3).astype(np.float32)
    gyk = np.broadcast_to(gyk, (128, NT, 9)).reshape(128, 144).copy()
    gxk = (r % 64 - 1 + kk % 3).astype(np.float32)
    gxk = np.broadcast_to(gxk, (128, NT, 9)).reshape(128, 144).copy()

    return {
        "xcl": xcl,
        "xcm": xcm,
        "woff": wof,
        "wcv": wc,
        "gyk": gyk,
        "gxk": gxk,
        "boff": b_off.reshape(27, 1).astype(np.float32),
        "identb": np.eye(128, dtype=np.float32).astype(BbArr),
        "identf": np.eye(32, dtype=np.float32),
    }


BbArr = ml_dtypes.bfloat16

_NC = None


def kernel(x, w_off, b_off, w_conv):
    global _NC
    x = np.asarray(x, np.float32)
    w_off = np.asarray(w_off, np.float32)
    b_off = np.asarray(b_off, np.float32)
    w_conv = np.asarray(w_conv, np.float32)
    if _NC is None:
        _NC = build_program()
    in_maps = [make_core_inputs(x, w_off, b_off, w_conv, c)
               for c in range(N_CORES)]
    res = run_bass_kernel_spmd(_NC, in_maps, core_ids=list(range(N_CORES)))
    out = np.empty((B, O, H, W), np.float32)
    for c in range(N_CORES):
        b, s = c // 2, c % 2
        out[b, :, 32 * s:32 * s + 32, :] = res.results[c]["y"].reshape(O, 32, W)
    return out



# revision 8
# speedup vs baseline: 4.0982x; 4.0982x over previous
"""Deformable Conv v2 (DCNv2) Trainium2 Bass kernel.

Problem: x[4,256,64,64], w_off[27,256,3,3], b_off[27], w_conv[256,256,3,3]
  -> out[4,256,64,64]  (offset conv + bilinear sampling + 9-point GEMM)

Sharding: 8 cores = 4 batches x 2 H-halves. Each core computes out for its
(batch, 32-row half): 2048 output pixels.

Per-core pipeline (single SPMD program):
  1. offset conv as 9 shifted-tap GEMMs on a C-major padded bf16 slice
  2. PE-transpose om to pixel-major, compute bilinear coefs (fp32) + indices
  3. dma_gather of 4C rows from a row-pair-interleaved channels-last image:
     xpair[y, x] holds rows (y, y+1) at column x, so one contiguous 2KB
     element covers all four bilinear corners (one descriptor per sample)
  4. per (k, pixel-tile): 4 tensor-scalar corner mults (bf16) + tree add
  5. main GEMM: out[o,p] += W_k[c,o].T @ val_k[c,p], 18 K-tiles, bf16
"""

import numpy as np
import ml_dtypes

import concourse.bacc as bacc
import concourse.bass as bass
import concourse.mybir as mybir
import concourse.tile as tile
from concourse.bass_utils import run_bass_kernel_spmd

F32 = mybir.dt.float32
BF16 = mybir.dt.bfloat16
I16 = mybir.dt.int16
OP = mybir.AluOpType
AF = mybir.ActivationFunctionType

B, C, H, W, O, K = 4, 256, 64, 64, 256, 9
PADR = 2                      # zero-pad ring width
Hp, Wp = H + 2 * PADR, W + 2 * PADR            # 68, 68
NPIX = 2048                   # output pixels per core (32 rows x 64)
NT = NPIX // 128              # 16 pixel tiles
N_CORES = 8

Bb = ml_dtypes.bfloat16

# om is computed for 32 output rows on full 68-wide (incl pad) columns
OMCOLS = 32 * Wp              # 2176
XCM_COLS = 36 * Wp            # 2448 (om conv input slice: rows h0..h0+35)
OM_BLOCKS = [(0, 512), (512, 512), (1024, 512), (1536, 512), (2048, 128)]


def build_program():
    nc = bacc.Bacc("TRN2", target_bir_lowering=False, debug=False,
                   num_devices=N_CORES)
    # row-pair interleaved channels-last image: [(Hp-1)*Wp, 2C]
    xcl_d = nc.dram_tensor("xcl", [(Hp - 1) * Wp * 2 * C], BF16,
                           kind="ExternalInput")
    xcm_d = nc.dram_tensor("xcm", [2, 128, XCM_COLS], BF16, kind="ExternalInput")
    woff_d = nc.dram_tensor("woff", [2, 128, 9 * 27], BF16, kind="ExternalInput")
    wcv_d = nc.dram_tensor("wcv", [2, 128, 2 * 9 * 128], BF16, kind="ExternalInput")
    gyk_d = nc.dram_tensor("gyk", [128, 144], F32, kind="ExternalInput")
    gxk_d = nc.dram_tensor("gxk", [128, 144], F32, kind="ExternalInput")
    boff_d = nc.dram_tensor("boff", [27, 1], F32, kind="ExternalInput")
    identb_d = nc.dram_tensor("identb", [128, 128], BF16, kind="ExternalInput")
    identf_d = nc.dram_tensor("identf", [32, 32], F32, kind="ExternalInput")
    y_d = nc.dram_tensor("y", [2, 128, NPIX], F32, kind="ExternalOutput")

    scratch_a = nc.dram_tensor("idx_scratch_a", [128 * 144], I16, kind="Internal")
    scratch_b = nc.dram_tensor("idx_scratch_b", [16 * 8 * 144], I16, kind="Internal")
    gidx_d = nc.dram_tensor("gidx", [128, 8], I16, kind="ExternalInput")

    with tile.TileContext(nc) as tc:
        _emit(nc, tc, xcl_d, xcm_d, woff_d, wcv_d, gyk_d, gxk_d, boff_d,
              identb_d, identf_d, y_d, scratch_a, scratch_b, gidx_d)
    nc.compile()
    return nc


def _emit(nc, tc, xcl_d, xcm_d, woff_d, wcv_d, gyk_d, gxk_d, boff_d,
          identb_d, identf_d, y_d, scratch_a, scratch_b, gidx_d):
    with tc.tile_pool(name="const", bufs=1) as cpool:
        _emit_body(nc, tc, cpool, xcl_d, xcm_d, woff_d, wcv_d, gyk_d, gxk_d,
                   boff_d, identb_d, identf_d, y_d, scratch_a, scratch_b,
                   gidx_d)


def _emit_body(nc, tc, cpool, xcl_d, xcm_d, woff_d, wcv_d, gyk_d, gxk_d,
               boff_d, identb_d, identf_d, y_d, scratch_a, scratch_b, gidx_d):
    # --- persistent constants ---
    wcv = [cpool.tile([128, 2 * 9 * 128], BF16, tag=f"wcv{ct}", name=f"wcv{ct}") for ct in range(2)]
    for ct in range(2):
        nc.sync.dma_start(wcv[ct][:, :], wcv_d.ap()[ct])
    woff = [cpool.tile([128, 9 * 27], BF16, tag=f"woff{ct}", name=f"woff{ct}") for ct in range(2)]
    for ct in range(2):
        nc.sync.dma_start(woff[ct][:, :], woff_d.ap()[ct])
    gyk = cpool.tile([128, 144], F32, tag="gyk", name="gyk")
    nc.sync.dma_start(gyk[:, :], gyk_d.ap()[:, :])
    gxk = cpool.tile([128, 144], F32, tag="gxk", name="gxk")
    nc.sync.dma_start(gxk[:, :], gxk_d.ap()[:, :])
    boff = cpool.tile([27, 1], F32, tag="boff", name="boff")
    nc.sync.dma_start(boff[:, :], boff_d.ap()[:, :])
    identb = cpool.tile([128, 128], BF16, tag="identb", name="identb")
    nc.sync.dma_start(identb[:, :], identb_d.ap()[:, :])
    identf = cpool.tile([32, 32], F32, tag="identf", name="identf")
    nc.sync.dma_start(identf[:, :], identf_d.ap()[:, :])

    # persistent: corner coefs, wrapped indices
    c00 = cpool.tile([128, 144], F32, tag="c00", name="c00")
    c01 = cpool.tile([128, 144], F32, tag="c01", name="c01")
    c10 = cpool.tile([128, 144], F32, tag="c10", name="c10")
    c11 = cpool.tile([128, 144], F32, tag="c11", name="c11")
    iw = cpool.tile([128, 8 * 144], I16, tag="iw", name="iw")
    gidx = cpool.tile([128, 8], I16, tag="gidx", name="gidx")
    nc.sync.dma_start(gidx[:, :], gidx_d.ap()[:, :])

    # ---------------- Phase A: offset conv + coefs (scoped pools) ---------
    with tc.tile_pool(name="early", bufs=1) as epool, \
         tc.tile_pool(name="om_ps", bufs=2, space="PSUM") as om_ps, \
         tc.tile_pool(name="omp_ps", bufs=1, space="PSUM") as omp_ps:
        xcm = [epool.tile([128, XCM_COLS], BF16, tag=f"xcm{ct}", name=f"xcm{ct}") for ct in range(2)]
        for ct in range(2):
            nc.sync.dma_start(xcm[ct][:, :], xcm_d.ap()[ct])

        om_s = epool.tile([27, OMCOLS], F32, tag="om_s", name="om_s")
        for nboff, nbsz in OM_BLOCKS:
            omp = om_ps.tile([27, 512], F32, tag="omps", name="omps")
            first = True
            for tap in range(9):
                ky, kx = tap // 3, tap % 3
                toff = (ky + 1) * Wp + kx - 1
                for ct in range(2):
                    nc.tensor.matmul(
                        omp[:, 0:nbsz],
                        woff[ct][:, tap * 27:(tap + 1) * 27],
                        xcm[ct][:, toff + nboff: toff + nboff + nbsz],
                        start=first, stop=(tap == 8 and ct == 1),
                    )
                    first = False
            nc.scalar.activation(om_s[:, nboff:nboff + nbsz], omp[:, 0:nbsz],
                                 AF.Identity, bias=boff[:, 0:1])

        # om -> pixel-major via PE transpose (compact valid pixels first:
        # matmul operands must have a single free dim)
        om_v = epool.tile([27, NPIX], F32, tag="om_v", name="om_v")
        om_h = om_s[:, :]
        nc.vector.tensor_copy(
            om_v[:, :],
            bass.AP(om_h.tensor, om_h.offset + 2,
                    [list(om_h.ap[0]), [Wp, 32], [1, 64]]))
        omp_pm = omp_ps.tile([128, NT * 27], F32, tag="omppm", name="omppm")
        for t in range(NT):
            nc.tensor.matmul(omp_pm[:, 27 * t:27 * (t + 1)],
                             om_v[:, 128 * t:128 * (t + 1)],
                             identf[0:27, 0:27], is_transpose=True,
                             start=True, stop=True)
        omp_s = epool.tile([128, NT * 27], F32, tag="omp_s", name="omp_s")
        nc.scalar.copy(omp_s[:, :], omp_pm[:, :])

        # --- coef pipeline (pixel-major [128, 16, 9] strided views) ---
        base = omp_s[:, :]
        p0 = list(base.ap[0])

        def omview(ch_off, ch_step):
            return bass.AP(base.tensor, base.offset + ch_off,
                           [p0, [27, NT], [ch_step, 9]])

        def wtile(tag):
            return epool.tile([128, 144], F32, tag=tag, name=tag)

        py = wtile("py")
        nc.vector.tensor_tensor(py[:, :], omview(0, 2), gyk[:, :], OP.add)
        px = wtile("px")
        nc.vector.tensor_tensor(px[:, :], omview(1, 2), gxk[:, :], OP.add)

        # floor via +16-bias cast roundtrip (correct for trunc OR round-to-
        # nearest casts; bias keeps the operand positive, clamp absorbs it).
        I32 = mybir.dt.int32
        BIAS = 16.0

        def floor_frac(p, pre):
            pt = wtile(pre + "t")
            nc.vector.tensor_scalar(pt[:, :], p[:, :], BIAS, None, OP.add)
            pi = epool.tile([128, 144], I32, tag=pre + "i", name=pre + "i")
            nc.vector.tensor_copy(pi[:, :], pt[:, :])
            pf = wtile(pre + "f")
            nc.vector.tensor_copy(pf[:, :], pi[:, :])
            gg = wtile(pre + "g")
            nc.vector.tensor_tensor(gg[:, :], pf[:, :], pt[:, :], OP.is_gt)
            fb = wtile(pre + "fb")   # floor(p)+BIAS
            nc.vector.tensor_tensor(fb[:, :], pf[:, :], gg[:, :], OP.subtract)
            fr = wtile(pre + "fr")   # frac(p)
            nc.vector.tensor_tensor(fr[:, :], pt[:, :], fb[:, :], OP.subtract)
            return fb, fr

        y0b, wy = floor_frac(py, "y")
        x0b, wx = floor_frac(px, "x")
        # clamp (still biased by +16): [-2, H] -> [14, H+16]
        nc.vector.tensor_scalar(y0b[:, :], y0b[:, :], 14.0, float(H) + BIAS,
                                OP.max, OP.min)
        nc.vector.tensor_scalar(x0b[:, :], x0b[:, :], 14.0, float(W) + BIAS,
                                OP.max, OP.min)
        # pair-space idx = 68*(y0+2) + x0+2 = 68*y0b + x0b - 966
        idxf = epool.tile([128, 144], F32, tag="idxf", name="idxf")
        nc.vector.tensor_scalar(idxf[:, :], y0b[:, :], float(Wp), -966.0,
                                OP.mult, OP.add)
        nc.vector.tensor_tensor(idxf[:, :], idxf[:, :], x0b[:, :], OP.add)
        idx16 = epool.tile([128, 144], I16, tag="idx16", name="idx16")
        nc.vector.tensor_copy(idx16[:, :], idxf[:, :])

        msk = wtile("msk")
        nc.scalar.activation(msk[:, :], omview(18, 1), AF.Sigmoid)
        b1 = wtile("b1")
        nc.vector.tensor_tensor(b1[:, :], wy[:, :], msk[:, :], OP.mult)
        b0 = wtile("b0")
        nc.vector.tensor_tensor(b0[:, :], msk[:, :], b1[:, :], OP.subtract)
        nc.vector.tensor_tensor(c01[:, :], b0[:, :], wx[:, :], OP.mult)
        nc.vector.tensor_tensor(c00[:, :], b0[:, :], c01[:, :], OP.subtract)
        nc.vector.tensor_tensor(c11[:, :], b1[:, :], wx[:, :], OP.mult)
        nc.vector.tensor_tensor(c10[:, :], b1[:, :], c11[:, :], OP.subtract)

        # --- index wrap to 16-partition layout, all coarse-grained DMA ---
        # 1. contiguous row dump: scratch_a[r*144 + k*16 + t] = idx(k,t,r)
        # 2. DRAM->DRAM permute: scratch_b[q*1152 + u*144 + c]
        #    = scratch_a[(16u+q)*144 + c]   (288B runs)
        # 3. 8x-replicating dma_gather (idx i%16) fills iw[p, u*144+k*16+t]
        idx16b = epool.tile([128, 144], I16, tag="idx16b", name="idx16b")
        nc.vector.tensor_copy(
            idx16b[:, :].rearrange("p (k t) -> p k t", t=NT),
            idx16[:, :].rearrange("p (t k) -> p k t", k=9))
        # contiguous row dump, then strided 288B-run replication reads:
        # iw[16rep+q, u*144+c] = scratch_a[(16u+q)*144 + c] = idx(.., r=16u+q)
        nc.sync.dma_start(
            scratch_a.ap().rearrange("(p c) -> p c", p=128),
            idx16b[:, :])
        for rep in range(8):
            nc.sync.dma_start(
                iw[16 * rep:16 * rep + 16, :].rearrange(
                    "p (u c) -> p u c", u=8),
                scratch_a.ap().rearrange("(u q c) -> q u c",
                                         u=8, q=16, c=144))

    # ---------------- Phase B: gather / apply / transpose / GEMM ----------
    # row-pair image: element at pair-idx i covers rows (y,y+1) x cols
    # (x, x+1) x C, contiguous 4C starting at offset 2C*i.
    winP = bass.AP(xcl_d, 0, [[2 * C, (Hp - 1) * Wp - 1], [1, 4 * C]])

    with tc.tile_pool(name="val", bufs=9) as vpool, \
         tc.tile_pool(name="g", bufs=2) as gpool, \
         tc.tile_pool(name="ab", bufs=4) as apool, \
         tc.tile_pool(name="outs", bufs=2) as opool, \
         tc.tile_pool(name="gemm_ps", bufs=4, space="PSUM") as gemm_ps, \
         tc.tile_pool(name="tp_ps", bufs=4, space="PSUM") as tp_ps:

        vals = []
        gps0 = [gemm_ps.tile([128, 512], F32, tag="gps", name="gps") for _ in range(4)]

        def emit_gemm_k(gps, k, ot):
            for ct in range(2):
                for nb in range(4):
                    nc.tensor.matmul(
                        gps[nb][:, :],
                        wcv[ct][:, (ot * 9 + k) * 128:(ot * 9 + k + 1) * 128],
                        vals[k][:, ct, nb * 512:(nb + 1) * 512],
                        start=(k == 0 and ct == 0),
                        stop=(k == 8 and ct == 1),
                    )

        for k in range(9):
            g4 = gpool.tile([128, NT, 4 * C], BF16, tag="g4", name="g4")
            ik = gpool.tile([128, 128], I16, tag="ik", name="ik")
            iw_ap = iw[:, :]
            nc.vector.tensor_copy(
                ik[:, :].rearrange("p (t u) -> p t u", u=8),
                bass.AP(iw_ap.tensor, iw_ap.offset + 16 * k,
                        [list(iw_ap.ap[0]), [1, 16], [144, 8]]))
            for q in range(4):   # 512-idx calls: tiles [4q, 4q+4)
                nc.gpsimd.dma_gather(
                    out_ap=g4[:, 4 * q:4 * q + 4, :], in_ap=winP,
                    idxs_ap=ik[:, 32 * q:32 * q + 32],
                    num_idxs=512, num_idxs_reg=512,
                    elem_size=4 * C, elem_step=2 * C)

            val = vpool.tile([128, 2, NPIX], BF16, tag="val", name="val")
            vals.append(val)
            for half in range(4):      # 4 pixel-quads of 4 tiles each
                tp = [tp_ps.tile([128, 512], BF16, tag="tp", name="tp") for _ in range(2)]
                for t in range(4 * half, 4 * half + 4):
                    col = t * 9 + k
                    # g4 element: [A=(y0,x0) | B=(y1,x0) | A'=(y0,x1) | B'=(y1,x1)]
                    mb = apool.tile([128, 4, C], BF16, tag="mb", name="mb")
                    nc.vector.tensor_scalar(mb[:, 0, :], g4[:, t, 0:C],
                                            c00[:, col:col + 1], None, OP.mult)
                    nc.scalar.activation(mb[:, 1, :], g4[:, t, C:2 * C],
                                         AF.Copy, scale=c10[:, col:col + 1])
                    nc.vector.tensor_scalar(mb[:, 2, :], g4[:, t, 2 * C:3 * C],
                                            c01[:, col:col + 1], None, OP.mult)
                    nc.vector.tensor_scalar(mb[:, 3, :], g4[:, t, 3 * C:4 * C],
                                            c11[:, col:col + 1], None, OP.mult)
                    ab = apool.tile([128, 2, C], BF16, tag="ab", name="ab")
                    nc.vector.tensor_tensor(ab[:, :, :], mb[:, 0:2, :],
                                            mb[:, 2:4, :], OP.add)
                    vt = apool.tile([128, C], BF16, tag="vt", name="vt")
                    nc.vector.tensor_tensor(vt[:, :], ab[:, 0, :],
                                            ab[:, 1, :], OP.add)
                    # PE transpose pixel-major val tile -> C-major (PSUM)
                    sl = slice((t % 4) * 128, (t % 4) * 128 + 128)
                    for ch in range(2):
                        nc.tensor.matmul(tp[ch][:, sl],
                                         vt[:, ch * 128:(ch + 1) * 128],
                                         identb[:, :], is_transpose=True,
                                         start=True, stop=True)
                for ch in range(2):
                    nc.scalar.copy(val[:, ch, half * 512:(half + 1) * 512],
                                   tp[ch][:, :])
            if k >= 1:
                emit_gemm_k(gps0, k - 1, ot=0)

        emit_gemm_k(gps0, 8, ot=0)
        out0 = opool.tile([128, NPIX], F32, tag="outs", name="outs")
        for nb in range(4):
            nc.scalar.copy(out0[:, nb * 512:(nb + 1) * 512], gps0[nb][:, :])
        nc.sync.dma_start(y_d.ap()[0], out0[:, :])

        gps1 = [gemm_ps.tile([128, 512], F32, tag="gps", name="gps") for _ in range(4)]
        for k in range(9):
            emit_gemm_k(gps1, k, ot=1)
        out1 = opool.tile([128, NPIX], F32, tag="outs", name="outs")
        for nb in range(4):
            nc.scalar.copy(out1[:, nb * 512:(nb + 1) * 512], gps1[nb][:, :])
        nc.sync.dma_start(y_d.ap()[1], out1[:, :])


# ---------------------------------------------------------------------------
# Host side
# ---------------------------------------------------------------------------

def make_core_inputs(x, w_off, b_off, w_conv, core):
    b, s = core // 2, core % 2
    h0 = 32 * s
    xp = np.zeros((C, Hp, Wp), np.float32)
    xp[:, PADR:PADR + H, PADR:PADR + W] = x[b]
    # channels-last padded image, then row-pair interleave:
    # xpair[y, x, :] = concat(xcl[y, x, :], xcl[y+1, x, :])
    xcl = np.ascontiguousarray(xp.transpose(1, 2, 0))          # [Hp, Wp, C]
    xpair = np.concatenate([xcl[:-1], xcl[1:]], axis=2)        # [Hp-1, Wp, 2C]
    xpair = np.ascontiguousarray(xpair).reshape(-1).astype(BbArr)
    xcm = np.ascontiguousarray(
        xp.reshape(C, Hp * Wp)[:, h0 * Wp: h0 * Wp + XCM_COLS]
    ).astype(BbArr).reshape(2, 128, XCM_COLS)

    # w_off [27, C, 3, 3] -> [2, 128, 9*27]: [ct, c, tap*27+oc]
    wof = w_off.transpose(1, 2, 3, 0).reshape(2, 128, 9, 27).reshape(2, 128, 243)
    wof = np.ascontiguousarray(wof).astype(BbArr)
    # w_conv [O, C, 3, 3] -> [2(ct), 128(c), (ot*9+k)*128+o]
    wc = w_conv.reshape(2, 128, C, 9).transpose(2, 0, 3, 1)   # [c, ot, k, o128]
    wc = np.ascontiguousarray(wc.reshape(2, 128, 2 * 9 * 128)).astype(BbArr)

    r = np.arange(128)[:, None, None]
    t = np.arange(NT)[None, :, None]
    kk = np.arange(9)[None, None, :]
    gyk = (h0 + 2 * t + r // 64 - 1 + kk // 3).astype(np.float32)
    gyk = np.broadcast_to(gyk, (128, NT, 9)).reshape(128, 144).copy()
    gxk = (r % 64 - 1 + kk % 3).astype(np.float32)
    gxk = np.broadcast_to(gxk, (128, NT, 9)).reshape(128, 144).copy()

    return {
        "xcl": xpair,
        "xcm": xcm,
        "woff": wof,
        "wcv": wc,
        "gyk": gyk,
        "gxk": gxk,
        "boff": b_off.reshape(27, 1).astype(np.float32),
        "identb": np.eye(128, dtype=np.float32).astype(BbArr),
        "identf": np.eye(32, dtype=np.float32),
        "gidx": np.broadcast_to((np.arange(128) % 16).astype(np.int16)[:, None],
                                (128, 8)).copy(),
    }


BbArr = ml_dtypes.bfloat16

_NC = None


def kernel(x, w_off, b_off, w_conv):
    global _NC
    x = np.asarray(x, np.float32)
    w_off = np.asarray(w_off, np.float32)
    b_off = np.asarray(b_off, np.float32)
    w_conv = np.asarray(w_conv, np.float32)
    if _NC is None:
        _NC = build_program()
    in_maps = [make_core_inputs(x, w_off, b_off, w_conv, c)
               for c in range(N_CORES)]
    res = run_bass_kernel_spmd(_NC, in_maps, core_ids=list(range(N_CORES)))
    out = np.empty((B, O, H, W), np.float32)
    for c in range(N_CORES):
        b, s = c // 2, c % 2
        out[b, :, 32 * s:32 * s + 32, :] = res.results[c]["y"].reshape(O, 32, W)
    return out
